# revision 25
# baseline (speedup 1.0000x reference)
"""Trainium2 Bass kernel for the BinaryLayer problem.

Math: out[b,o] = OR_r ( mask[o,r] AND AND_t x_in[b, w[o,r,t]] ) with
x_in = [1 | x | 1-x].  AND over 16 literals == (sum of literal values == 16).
sum_t lit = base[j] + sum_f C[f,j]*x[b,f]  where for j=(o,r):
  C[f,j]  = (#slots with w==f+1) - (#slots with w==f+1+F)
  base[j] = (#slots with w==0) + (#slots with w>F)
Fold threshold+mask into a constant row: c1[j] = base[j]-16 (valid term)
or base[j]-64 (padded term, all w==0).  Then with S[b,j] = x_aug[b,:]@A[:,j]
(A = [C; c1], x_aug = [x, 1]):  AND true <=> S==0, and since S<=0 always,
out[b,o] = (max_r S[b,o*32+r] >= 0).  All arithmetic is exact small-int
in fp8e4m3 inputs / f32 PSUM accumulation.

Sharding: data-parallel over batch B across 8 cores; A replicated.

Device layout: k lives on [partition p, subtile s] with k = s*128 + p.
x^T is shipped unpadded [785, BS]; the tail subtile s=6 is filled with an
overlapping DMA of x rows 657..784 (so no memset is needed), and the A
table - shipped padded to 896 rows - carries zeros for the duplicated
rows so they contribute nothing.
"""

import os

os.environ.setdefault("MYCRO_LOCAL_CACHE", "1")

import numpy as np
import ml_dtypes

import concourse.bass as bass
import concourse.bacc as bacc
import concourse.mybir as mybir
from concourse.tile import TileContext
from concourse.bass_utils import run_bass_kernel_spmd

B, F = 4096, 784
OUT, OR_T, AND_T = 128, 32, 16
N_CORES = 8
BS = B // N_CORES            # 512 batch rows per core
K = F + 1                    # 785 = 784 features + constant row
KFULL = 6                    # full 128-row k-subtiles
KSUB = 8                     # 6 full + 17-row tail + 1 zero pad (for a uniform DR tail pair)
KTAIL = K - KFULL * 128      # 17 real rows in the tail k-subtile
J = OUT * OR_T               # 4096 (o,r) columns, j = o*32 + r
NBLK = 512                   # one f32 PSUM bank
NJB = J // NBLK              # 8
NBT = BS // 128              # 4 batch tiles per core
FP8 = mybir.dt.float8e4
FP8_NP = mybir.dt.np(FP8)

_CACHE: dict = {}


def _build_nc(use_double_row: bool) -> bass.Bass:
    nc = bacc.Bacc("TRN2")
    xT_d = nc.declare_dram_parameter("xT", [K, BS], mybir.dt.int32, isOutput=False)
    A_d = nc.declare_dram_parameter("A", [K, J], FP8, isOutput=False)
    y_d = nc.declare_dram_parameter("y", [BS, OUT], mybir.dt.uint8, isOutput=True)

    with TileContext(nc) as tc:
        with (
            tc.tile_pool(name="const", bufs=1) as cpool,
            tc.tile_pool(name="psum", bufs=4, space="PSUM") as ppool,
            tc.tile_pool(name="outp", bufs=4) as opool,
        ):
            # A in SBUF: [p, s, j] fp8 with k = s*128 + p.
            # x^T (with ones row) in SBUF, same [p, s, b] layout; the tail
            # subtile s=6 holds x rows 657..784 (overlapping s=5; the
            # duplicated rows carry zero coefficients in A).
            #
            # Loads are chunked - x per k-subtile, A per (k-subtile,
            # J-quarter) - and interleaved in consumption order so the
            # matmul stream starts a couple microseconds in and is never
            # DMA-gated.  Compute runs in 4 J-quarter rounds: per round the
            # 8 PSUM banks are (4 batch tiles) x (2 j-blocks), fully
            # accumulated over K then max-reduced by DVE while PE moves on
            # to the next quarter.
            # Round structure over J: two 512-col eighths first (so the
            # first PSUM banks complete as early as possible), then three
            # 1024-col quarters.  Column offset of round r:
            # Rounds: (batch-tile pair) x (1024-col J-quarter) = 4 PSUM
            # banks, so two rounds fit in PSUM and PE/DVE stream without
            # lockstep.  Quarter-major order so each A chunk feeds two
            # consecutive rounds.
            NQ = 4
            QW = J // NQ
            A_sb = cpool.tile([128, KSUB, J], FP8)
            x_i = cpool.tile([128, KSUB, BS], mybir.dt.int32)
            x_q = cpool.tile([128, KSUB, BS], FP8)

            # PE warm-up scratch: the tensor engine needs ~3us of sustained
            # work to reach full clock, so a few dummy matmuls on zeroed
            # scratch run while the first x/A chunks are still in flight.
            wu_sb = cpool.tile([128, 640], FP8)
            nc.gpsimd.memset(wu_sb[:], 0.0)

            # Tail k-subtiles: s=6 has only 17 real rows (k=768..784), s=7 is
            # all zero.  Zero them on the otherwise-idle gpsimd engine before
            # the 17-row DMAs land on partitions 0..16; A's (large) region is
            # zeroed per J-quarter so the first quarter is ready early.
            nc.gpsimd.memset(x_i[:, KFULL : KFULL + 2, :], 0)
            for q in range(4):
                nc.gpsimd.memset(A_sb[:, KFULL : KFULL + 2, q * 1024 : (q + 1) * 1024], 0.0)

            # Every DMA costs ~625ns on the (shared) HWDGE issue ring, so
            # loads are batched into few chunks, issued in consumption
            # order: all of x first (k-subtile pairs, casts chasing each),
            # then A as (k-subtile-pair x round-columns) chunks, round by
            # round.
            # Round-0 feeds PE as early as possible: x subtile-pairs
            # interleaved with the matching round-0 A chunks, casts chasing
            # each x chunk on alternating ACT/DVE (both idle this early).
            e0 = slice(0, QW)

            def A_pair(k, jsl):
                nc.sync.dma_start(
                    out=A_sb[:, 2 * k : 2 * k + 2, jsl],
                    in_=A_d[256 * k : 256 * (k + 1), jsl].rearrange(
                        "(s p) j -> p s j", p=128
                    ),
                )

            def x_one(s):
                nc.sync.dma_start(
                    out=x_i[:, s, :], in_=xT_d[s * 128 : (s + 1) * 128, :]
                )

            x_one(0)
            x_one(1)
            nc.sync.dma_start(out=x_i[0:KTAIL, KFULL, :], in_=xT_d[KFULL * 128 : K, :])
            A_pair(0, e0)
            nc.sync.dma_start(out=A_sb[0:KTAIL, KFULL, e0], in_=A_d[KFULL * 128 : K, e0])
            x_one(2)
            x_one(3)
            A_pair(1, e0)
            x_one(4)
            x_one(5)
            A_pair(2, e0)
            nc.scalar.copy(out=x_q[:, 0, :], in_=x_i[:, 0, :])
            nc.vector.tensor_copy(out=x_q[:, 1, :], in_=x_i[:, 1, :])
            nc.vector.tensor_copy(out=x_q[:, KFULL : KFULL + 2, :], in_=x_i[:, KFULL : KFULL + 2, :])
            nc.scalar.copy(out=x_q[:, 2, :], in_=x_i[:, 2, :])
            nc.vector.tensor_copy(out=x_q[:, 3, :], in_=x_i[:, 3, :])
            nc.scalar.copy(out=x_q[:, 4, :], in_=x_i[:, 4, :])
            nc.vector.tensor_copy(out=x_q[:, 5, :], in_=x_i[:, 5, :])

            for q in range(1, NQ):
                jsl = slice(q * QW, (q + 1) * QW)
                nc.sync.dma_start(
                    out=A_sb[:, 0:KFULL, jsl],
                    in_=A_d[0 : KFULL * 128, jsl].rearrange("(s p) j -> p s j", p=128),
                )
                nc.sync.dma_start(
                    out=A_sb[0:KTAIL, KFULL, jsl], in_=A_d[KFULL * 128 : K, jsl]
                )

            y_fs = [
                opool.tile([128, NJB, 16], mybir.dt.float32, name=f"y_f{bt}", tag=f"y_f{bt}")
                for bt in range(NBT)
            ]
            NR = NQ * 2
            for r in range(NR):
                q, bp = r // 2, r % 2
                bts = [2 * bp, 2 * bp + 1]
                banks = {}
                for bt in bts:
                    banks[bt] = ppool.tile(
                        [128, 2, 16, 32], mybir.dt.float32, name="ps", tag="ps"
                    )
                if r == 0:
                    for _ in range(8):
                        nc.tensor.matmul(
                            banks[bts[0]][:, 0],
                            wu_sb[:, 0:128],
                            wu_sb[:, 128:640],
                            start=True,
                            stop=True,
                        )
                n_sp = 4 if use_double_row else KSUB
                for sp in range(n_sp):
                    for bt in bts:
                        bsl = slice(bt * 128, (bt + 1) * 128)
                        for jq in range(2):
                            jsl = slice(
                                q * QW + jq * NBLK, q * QW + (jq + 1) * NBLK
                            )
                            if use_double_row:
                                ssl = slice(2 * sp, 2 * sp + 2)
                                nc.tensor.matmul(
                                    banks[bt][:, jq],
                                    x_q[:, ssl, bsl],
                                    A_sb[:, ssl, jsl],
                                    start=(sp == 0),
                                    stop=(sp == n_sp - 1),
                                    perf_mode=mybir.MatmulPerfMode.DoubleRow,
                                )
                            else:
                                nc.tensor.matmul(
                                    banks[bt][:, jq],
                                    x_q[:, sp, bsl],
                                    A_sb[:, sp, jsl],
                                    start=(sp == 0),
                                    stop=(sp == n_sp - 1),
                                )
                for bt in bts:
                    nc.vector.tensor_reduce(
                        out=y_fs[bt][:, q * 2 : q * 2 + 2, :],
                        in_=banks[bt][:],
                        axis=mybir.AxisListType.X,
                        op=mybir.AluOpType.max,
                    )
                    if q == NQ - 1:
                        # Final compare on the (idle) scalar engine: y values
                        # are integers <= 0 with 0 == True, so
                        # relu(y + 1) is exactly the 0/1 indicator.
                        bsl = slice(bt * 128, (bt + 1) * 128)
                        y_u = opool.tile(
                            [128, OUT], mybir.dt.uint8, name="y_u", tag="y_u"
                        )
                        nc.scalar.activation(
                            out=y_u[:],
                            in_=y_fs[bt][:],
                            func=mybir.ActivationFunctionType.Relu,
                            bias=1.0,
                            scale=1.0,
                        )
                        nc.sync.dma_start(out=y_d[bsl, :], in_=y_u[:])
    return nc


def _get_nc() -> bass.Bass:
    if "nc" not in _CACHE:
        nc = _build_nc(use_double_row=_CACHE.get("dr", True))
        nc.finalize()
        _CACHE["nc"] = nc
    return _CACHE["nc"]


def _build_A(weights: np.ndarray) -> np.ndarray:
    w = weights.reshape(J, AND_T).astype(np.int64)
    v = w.reshape(-1)
    j_idx = np.repeat(np.arange(J), AND_T)
    C = np.zeros((K, J), np.float32)
    pos = (v >= 1) & (v <= F)
    neg = v > F
    np.add.at(C, (v[pos] - 1, j_idx[pos]), 1.0)
    np.add.at(C, (v[neg] - 1 - F, j_idx[neg]), -1.0)
    base = (w == 0).sum(1) + neg.reshape(J, AND_T).sum(1)
    padded = (w == 0).all(1)
    C[F, :] = np.where(padded, base - 64.0, base - 16.0).astype(np.float32)
    A8 = C.astype(FP8_NP)
    assert np.array_equal(A8.astype(np.float32), C), "fp8 must be exact"
    return A8


def kernel(x: np.ndarray, weights: np.ndarray) -> np.ndarray:
    x = np.asarray(x)
    weights = np.asarray(weights)
    A8 = _build_A(weights)
    xT = np.concatenate(
        [np.ascontiguousarray(x.T).astype(np.int32), np.ones((1, B), np.int32)], axis=0
    )
    in_maps = [
        {"xT": np.ascontiguousarray(xT[:, c * BS : (c + 1) * BS]), "A": A8}
        for c in range(N_CORES)
    ]
    nc = _get_nc()
    res = run_bass_kernel_spmd(nc, in_maps, list(range(N_CORES)))
    y = np.concatenate([res.results[c]["y"] for c in range(N_CORES)], axis=0)
    return y.astype(bool)


# revision 27
# speedup vs baseline: 1.0099x; 1.0099x over previous
"""Trainium2 Bass kernel for the BinaryLayer problem.

Math: out[b,o] = OR_r ( mask[o,r] AND AND_t x_in[b, w[o,r,t]] ) with
x_in = [1 | x | 1-x].  AND over 16 literals == (sum of literal values == 16).
sum_t lit = base[j] + sum_f C[f,j]*x[b,f]  where for j=(o,r):
  C[f,j]  = (#slots with w==f+1) - (#slots with w==f+1+F)
  base[j] = (#slots with w==0) + (#slots with w>F)
Fold threshold+mask into a constant row: c1[j] = base[j]-16 (valid term)
or base[j]-64 (padded term, all w==0).  Then with S[b,j] = x_aug[b,:]@A[:,j]
(A = [C; c1], x_aug = [x, 1]):  AND true <=> S==0, and since S<=0 always,
out[b,o] = (max_r S[b,o*32+r] >= 0).  All arithmetic is exact small-int
in fp8e4m3 inputs / f32 PSUM accumulation.

Sharding: data-parallel over batch B across 8 cores; A replicated.

Device layout: k lives on [partition p, subtile s] with k = s*128 + p,
8 subtiles (6 full, a 17-row tail on partitions 0..16 of s=6, and an
all-zero s=7 so the tail runs as a DoubleRow pair too).  x^T and A ship
unpadded [785, *]; tail subtiles are zeroed on gpsimd.  All matmuls are
fp8 DoubleRow (warm: 107ns per 512-col matmul); a few dummy matmuls on
zeroed scratch pre-warm the PE clock while the first chunks load.
Compute runs in (batch-tile-pair x column-block) rounds of <=4 PSUM
banks so two rounds are always in flight; DVE max-reduces each bank
pair while PE streams on, and the final compare is relu(y+1) on ACT.
"""

import os

os.environ.setdefault("MYCRO_LOCAL_CACHE", "1")

import numpy as np
import ml_dtypes

import concourse.bass as bass
import concourse.bacc as bacc
import concourse.mybir as mybir
from concourse.tile import TileContext
from concourse.bass_utils import run_bass_kernel_spmd

B, F = 4096, 784
OUT, OR_T, AND_T = 128, 32, 16
N_CORES = 8
BS = B // N_CORES            # 512 batch rows per core
K = F + 1                    # 785 = 784 features + constant row
KFULL = 6                    # full 128-row k-subtiles
KSUB = 8                     # 6 full + 17-row tail + 1 zero pad (for a uniform DR tail pair)
KTAIL = K - KFULL * 128      # 17 real rows in the tail k-subtile
J = OUT * OR_T               # 4096 (o,r) columns, j = o*32 + r
NBLK = 512                   # one f32 PSUM bank
NJB = J // NBLK              # 8
NBT = BS // 128              # 4 batch tiles per core
FP8 = mybir.dt.float8e4
FP8_NP = mybir.dt.np(FP8)

_CACHE: dict = {}


def _build_nc(use_double_row: bool) -> bass.Bass:
    nc = bacc.Bacc("TRN2")
    xT_d = nc.declare_dram_parameter("xT", [K, BS], mybir.dt.int32, isOutput=False)
    A_d = nc.declare_dram_parameter("A", [K, J], FP8, isOutput=False)
    y_d = nc.declare_dram_parameter("y", [BS, OUT], mybir.dt.uint8, isOutput=True)

    with TileContext(nc) as tc:
        with (
            tc.tile_pool(name="const", bufs=1) as cpool,
            tc.tile_pool(name="psum", bufs=4, space="PSUM") as ppool,
            tc.tile_pool(name="outp", bufs=4) as opool,
        ):
            # A and x^T in SBUF as [p, s, cols] fp8 with k = s*128 + p.
            # Rounds: (batch-tile pair) x (column block) of at most 2 PSUM
            # banks per batch tile, so two rounds fit in PSUM and PE/DVE
            # stream without lockstep.  The first two column blocks are
            # 512-col eighths so the first banks complete on much less A
            # data; the rest are 1024-col quarters (fewer, cheaper reduces).
            COLS = [(0, 512), (512, 512), (1024, 1024), (2048, 1024), (3072, 1024)]
            A_sb = cpool.tile([128, KSUB, J], FP8)
            x_i = cpool.tile([128, KSUB, BS], mybir.dt.int32)
            x_q = cpool.tile([128, KSUB, BS], FP8)

            # PE warm-up scratch: the tensor engine needs ~3us of sustained
            # work to reach full clock, so a few dummy matmuls on zeroed
            # scratch run while the first x/A chunks are still in flight.
            wu_sb = cpool.tile([128, 640], FP8)
            nc.gpsimd.memset(wu_sb[:], 0.0)

            # Tail k-subtiles: s=6 has only 17 real rows (k=768..784), s=7 is
            # all zero.  Zero them on the otherwise-idle gpsimd engine before
            # the 17-row DMAs land on partitions 0..16; A's (large) region is
            # zeroed per J-quarter so the first quarter is ready early.
            nc.gpsimd.memset(x_i[:, KFULL : KFULL + 2, :], 0)
            for q in range(4):
                nc.gpsimd.memset(A_sb[:, KFULL : KFULL + 2, q * 1024 : (q + 1) * 1024], 0.0)

            # Every DMA costs ~625ns of shared HWDGE issue-ring time and
            # ~2.4us completion-receipt latency, so loads are few, sized to
            # need, and issued in consumption order: x subtiles interleaved
            # with the (small) first-column-block A chunks so PE starts
            # early, casts chasing each x chunk on alternating ACT/DVE.
            e0 = slice(0, 512)

            def A_pair(k, jsl):
                nc.sync.dma_start(
                    out=A_sb[:, 2 * k : 2 * k + 2, jsl],
                    in_=A_d[256 * k : 256 * (k + 1), jsl].rearrange(
                        "(s p) j -> p s j", p=128
                    ),
                )

            def x_one(s):
                nc.sync.dma_start(
                    out=x_i[:, s, :], in_=xT_d[s * 128 : (s + 1) * 128, :]
                )

            x_one(0)
            x_one(1)
            nc.sync.dma_start(out=x_i[0:KTAIL, KFULL, :], in_=xT_d[KFULL * 128 : K, :])
            A_pair(0, e0)
            nc.sync.dma_start(out=A_sb[0:KTAIL, KFULL, e0], in_=A_d[KFULL * 128 : K, e0])
            x_one(2)
            x_one(3)
            A_pair(1, e0)
            x_one(4)
            x_one(5)
            A_pair(2, e0)
            nc.scalar.copy(out=x_q[:, 0, :], in_=x_i[:, 0, :])
            nc.vector.tensor_copy(out=x_q[:, 1, :], in_=x_i[:, 1, :])
            nc.vector.tensor_copy(out=x_q[:, KFULL : KFULL + 2, :], in_=x_i[:, KFULL : KFULL + 2, :])
            nc.scalar.copy(out=x_q[:, 2, :], in_=x_i[:, 2, :])
            nc.vector.tensor_copy(out=x_q[:, 3, :], in_=x_i[:, 3, :])
            nc.scalar.copy(out=x_q[:, 4, :], in_=x_i[:, 4, :])
            nc.vector.tensor_copy(out=x_q[:, 5, :], in_=x_i[:, 5, :])

            for off, w in COLS[1:]:
                jsl = slice(off, off + w)
                nc.sync.dma_start(
                    out=A_sb[:, 0:KFULL, jsl],
                    in_=A_d[0 : KFULL * 128, jsl].rearrange("(s p) j -> p s j", p=128),
                )
                nc.sync.dma_start(
                    out=A_sb[0:KTAIL, KFULL, jsl], in_=A_d[KFULL * 128 : K, jsl]
                )

            y_fs = [
                opool.tile([128, NJB, 16], mybir.dt.float32, name=f"y_f{bt}", tag=f"y_f{bt}")
                for bt in range(NBT)
            ]
            NR = len(COLS) * 2
            for r in range(NR):
                ci, bp = r // 2, r % 2
                off, w = COLS[ci]
                nbk = w // NBLK
                bts = [2 * bp, 2 * bp + 1]
                banks = {}
                for bt in bts:
                    banks[bt] = ppool.tile(
                        [128, nbk, 16, 32], mybir.dt.float32, name="ps", tag="ps"
                    )
                if r == 0:
                    for _ in range(8):
                        nc.tensor.matmul(
                            banks[bts[0]][:, 0],
                            wu_sb[:, 0:128],
                            wu_sb[:, 128:640],
                            start=True,
                            stop=True,
                        )
                n_sp = 4 if use_double_row else KSUB
                for sp in range(n_sp):
                    for bt in bts:
                        bsl = slice(bt * 128, (bt + 1) * 128)
                        for jq in range(nbk):
                            jsl = slice(
                                off + jq * NBLK, off + (jq + 1) * NBLK
                            )
                            if use_double_row:
                                ssl = slice(2 * sp, 2 * sp + 2)
                                nc.tensor.matmul(
                                    banks[bt][:, jq],
                                    x_q[:, ssl, bsl],
                                    A_sb[:, ssl, jsl],
                                    start=(sp == 0),
                                    stop=(sp == n_sp - 1),
                                    perf_mode=mybir.MatmulPerfMode.DoubleRow,
                                )
                            else:
                                nc.tensor.matmul(
                                    banks[bt][:, jq],
                                    x_q[:, sp, bsl],
                                    A_sb[:, sp, jsl],
                                    start=(sp == 0),
                                    stop=(sp == n_sp - 1),
                                )
                jf0 = off // NBLK
                for bt in bts:
                    nc.vector.tensor_reduce(
                        out=y_fs[bt][:, jf0 : jf0 + nbk, :],
                        in_=banks[bt][:],
                        axis=mybir.AxisListType.X,
                        op=mybir.AluOpType.max,
                    )
                    if ci == len(COLS) - 1:
                        # Final compare on the (idle) scalar engine: y values
                        # are integers <= 0 with 0 == True, so
                        # relu(y + 1) is exactly the 0/1 indicator.
                        bsl = slice(bt * 128, (bt + 1) * 128)
                        y_u = opool.tile(
                            [128, OUT], mybir.dt.uint8, name="y_u", tag="y_u"
                        )
                        nc.scalar.activation(
                            out=y_u[:],
                            in_=y_fs[bt][:],
                            func=mybir.ActivationFunctionType.Relu,
                            bias=1.0,
                            scale=1.0,
                        )
                        nc.sync.dma_start(out=y_d[bsl, :], in_=y_u[:])
    return nc


def _get_nc() -> bass.Bass:
    if "nc" not in _CACHE:
        nc = _build_nc(use_double_row=_CACHE.get("dr", True))
        nc.finalize()
        _CACHE["nc"] = nc
    return _CACHE["nc"]


def _build_A(weights: np.ndarray) -> np.ndarray:
    w = weights.reshape(J, AND_T).astype(np.int64)
    v = w.reshape(-1)
    j_idx = np.repeat(np.arange(J), AND_T)
    C = np.zeros((K, J), np.float32)
    pos = (v >= 1) & (v <= F)
    neg = v > F
    np.add.at(C, (v[pos] - 1, j_idx[pos]), 1.0)
    np.add.at(C, (v[neg] - 1 - F, j_idx[neg]), -1.0)
    base = (w == 0).sum(1) + neg.reshape(J, AND_T).sum(1)
    padded = (w == 0).all(1)
    C[F, :] = np.where(padded, base - 64.0, base - 16.0).astype(np.float32)
    A8 = C.astype(FP8_NP)
    assert np.array_equal(A8.astype(np.float32), C), "fp8 must be exact"
    return A8


def kernel(x: np.ndarray, weights: np.ndarray) -> np.ndarray:
    x = np.asarray(x)
    weights = np.asarray(weights)
    A8 = _build_A(weights)
    xT = np.concatenate(
        [np.ascontiguousarray(x.T).astype(np.int32), np.ones((1, B), np.int32)], axis=0
    )
    in_maps = [
        {"xT": np.ascontiguousarray(xT[:, c * BS : (c + 1) * BS]), "A": A8}
        for c in range(N_CORES)
    ]
    nc = _get_nc()
    res = run_bass_kernel_spmd(nc, in_maps, list(range(N_CORES)))
    y = np.concatenate([res.results[c]["y"] for c in range(N_CORES)], axis=0)
    return y.astype(bool)
